# revision 9
# baseline (speedup 1.0000x reference)
"""Self-contained Bass/Trainium2 kernel for nn_BasicGCN (4x GCNConv + Set2Set + MLP).

Distribution: nodes sharded by graph across 8 cores (graph-level data parallelism).
Per conv: local xw~ = (dinv*h) @ W computed on PE, fp16-AllGathered in 4 row-chunks
(each chunk tensor = one int16-addressable dma_gather window), per-edge gather via
dma_gather into degree-sorted padded [128-dest, slots, F] tiles, DVE slot-reduce,
dma_scatter_add combine of the 4 chunk-partials into an fp16 accumulator, then
per-partition dinv scale + bias + relu epilogue. Set2Set runs per-graph with graphs
on partitions (uniform padded slot layout); LSTM + MLP head on PE/ACT/DVE.
"""
import os
import numpy as np

# ---------------- configuration ----------------
F = 128          # feature dim (H == EMB)
NCORES = 8
NCONVS = 4
STEPS = 2
NEMB = 118


class Cfg:
    def __init__(self, N, E, B, NLOC, slot_cap=48):
        assert NLOC % 2048 == 0
        self.N, self.E, self.B = N, E, B
        self.NLOC = NLOC                 # padded local node rows (mult of 2048)
        self.CHUNK = NLOC // 4           # local rows per AG chunk
        self.TROWS = 8 * self.CHUNK      # data rows per chunk table
        assert self.TROWS + 128 <= 32768  # int16 gather index limit
        self.GPC = B // NCORES           # graphs per core
        self.G = NLOC // 128             # dest groups per core
        self.NSLAB = NLOC // 512         # epilogue slabs (4 groups each)
        self.SLABS_PER_CHUNK = self.CHUNK // 512
        self.slot_cap = slot_cap


FULL = Cfg(N=100000, E=1600000, B=1000, NLOC=14336)

LAST_EXEC_NS = None


def _install_ntff_hook():
    """Best-effort: make run_bass_kernel_spmd(trace=True) work under axon."""
    import contextlib, ctypes, sys, types
    try:
        import antenv.axon_hooks  # noqa: F401
        return
    except ImportError:
        pass
    cands = []
    try:
        for line in open("/proc/self/maps"):
            if "libaxon_pjrt.so" in line:
                cands.append(line.split()[-1])
                break
    except OSError:
        pass
    cands.append("/opt/axon/libaxon_pjrt.so")
    lib = None
    for so_path in cands:
        if not os.path.exists(so_path):
            continue
        try:
            cand = ctypes.CDLL(so_path)
        except OSError:
            continue
        if hasattr(cand, "axon_start_nrt_profile"):
            lib = cand
            break
    if lib is None:
        return
    lib.axon_start_nrt_profile.argtypes = [ctypes.POINTER(ctypes.c_int64), ctypes.c_size_t]
    lib.axon_start_nrt_profile.restype = ctypes.c_int64
    lib.axon_stop_nrt_profile.argtypes = [ctypes.c_char_p]
    lib.axon_stop_nrt_profile.restype = ctypes.c_int64

    @contextlib.contextmanager
    def _hook(output_dir, device_ids):
        import jax
        jax.devices()
        if device_ids:
            ids = (ctypes.c_int64 * len(device_ids))(*device_ids)
            rc = lib.axon_start_nrt_profile(ids, len(device_ids))
        else:
            rc = lib.axon_start_nrt_profile(None, 0)
        if rc != 0:
            raise RuntimeError(f"axon_start_nrt_profile rc={rc}")
        try:
            yield
        finally:
            lib.axon_stop_nrt_profile(str(output_dir).encode())

    mod = types.ModuleType("antenv.axon_hooks")
    mod.get_axon_ntff_profile_hook = lambda: _hook
    mod.set_axon_ntff_profile_hook = lambda h: None
    sys.modules["antenv.axon_hooks"] = mod
    try:
        import antenv
        antenv.axon_hooks = mod
    except ImportError:
        pass
    try:
        from concourse import bass_utils
        bass_utils.upload_artifacts = lambda tmpdir: f"local://{tmpdir}"
    except ImportError:
        pass


# ---------------- host-side preprocessing ----------------

def _wrap16(a):
    """int array [n] (n % 16 == 0) -> int16 [128, n/16] wrapped+replicated layout."""
    a = np.asarray(a)
    n = a.shape[0]
    assert n % 16 == 0
    assert a.min() >= 0 and a.max() < 32768, (a.min(), a.max())
    return np.tile(a.reshape(n // 16, 16).T, (8, 1)).astype(np.int16)


def _prep(cfg, x, edge_index, batch, emb, conv_W, conv_b,
          lstm_Wih, lstm_Whh, lstm_bih, lstm_bhh,
          lin_W0, lin_b0, lin_W1, lin_b1, lin3_W, lin3_b):
    N, B, NLOC, CHUNK, GPC = cfg.N, cfg.B, cfg.NLOC, cfg.CHUNK, cfg.GPC
    x = np.asarray(x).astype(np.int64)
    batch = np.asarray(batch).astype(np.int64)
    row = np.asarray(edge_index[0]).astype(np.int64)
    col = np.asarray(edge_index[1]).astype(np.int64)

    # node -> core via graph id (batch is sorted)
    nstart = np.searchsorted(batch, np.arange(NCORES) * GPC)
    nstart = np.concatenate([nstart, [N]])
    nloc = np.diff(nstart)
    assert nloc.max() <= NLOC, (nloc.max(), NLOC)
    kof = np.repeat(np.arange(NCORES), nloc)       # core of node
    loc = np.arange(N) - nstart[kof]               # local index of node

    deg = np.bincount(col, minlength=N).astype(np.float64) + 1.0
    dinv = (deg ** -0.5).astype(np.float32)

    # ---- conv gather structures ----
    # all slots: real edges + self loops
    e_dst = np.concatenate([col, np.arange(N)])
    e_src = np.concatenate([row, np.arange(N)])
    kd = kof[e_dst]
    dl = loc[e_dst].astype(np.int64)
    sc = kof[e_src]
    sl = loc[e_src].astype(np.int64)
    ch = sl // CHUNK                               # AG chunk of the source
    srel = sc * CHUNK + (sl - ch * CHUNK)          # row within chunk table
    ZREL = cfg.TROWS                               # zero row in each chunk table

    # per (core, chunk) slot counts per local dest
    cnts = np.zeros((NCORES, 4, NLOC), np.int64)
    np.add.at(cnts, (kd, ch, dl), 1)

    # common schedule: per chunk, group count and per-group slot counts
    sched = []        # sched[c] = list of (g_start, k, S) gather batches
    g_counts = []     # g_counts[c] = number of 128-dest groups (common)
    s_per_group = []  # s_per_group[c] = np.array [G_c] of slot counts
    for c in range(4):
        percore_sorted = [np.sort(cnts[k, c])[::-1] for k in range(NCORES)]
        ndst = max(int((s > 0).sum()) for s in percore_sorted)
        Gc = max(1, -(-ndst // 128))
        smax = np.zeros(Gc, np.int64)
        for k in range(NCORES):
            s = percore_sorted[k]
            v = s[np.arange(Gc) * 128]             # group max = first of each 128-run
            smax = np.maximum(smax, v)
        smax = np.maximum(smax, 1)
        g_counts.append(Gc)
        s_per_group.append(smax)
        # batches: consecutive equal-S groups, k*S <= slot_cap
        batches = []
        g = 0
        while g < Gc:
            S = int(smax[g])
            k = 1
            while (g + k < Gc and int(smax[g + k]) == S
                   and (k + 1) * S <= max(cfg.slot_cap, S)):
                k += 1
            batches.append((g, k, S))
            g += k
        sched.append(batches)

    # per-core idx / scatter streams
    per_core = []
    order_e = np.lexsort((e_src, dl, kd))  # stable grouping helper (not strictly needed)
    del order_e
    for k in range(NCORES):
        core = {}
        # x + dinv
        xl = np.zeros(NLOC, np.int64)
        xl[: nloc[k]] = x[nstart[k]: nstart[k + 1]]
        core["x16"] = _wrap16(xl)
        dv = np.zeros(NLOC, np.float32)
        dv[: nloc[k]] = dinv[nstart[k]: nstart[k + 1]]
        core["dinv"] = np.ascontiguousarray(dv.reshape(cfg.G, 128).T)  # [128, G]
        per_core.append(core)

    for c in range(4):
        Gc = g_counts[c]
        smax = s_per_group[c]
        base = np.concatenate([[0], np.cumsum(smax)])
        totslots = int(base[-1]) * 128
        for k in range(NCORES):
            m = (kd == k) & (ch == c)
            dsts = dl[m]
            srcs = srel[m]
            cnt = cnts[k, c]
            # rank dests by (count desc, dest asc)
            orderd = np.lexsort((np.arange(NLOC), -cnt))
            rank_of = np.empty(NLOC, np.int64)
            rank_of[orderd] = np.arange(NLOC)
            r = rank_of[dsts]
            # position of each slot within its dest
            o = np.lexsort((srcs, r))
            r_s, src_s = r[o], srcs[o]
            j = np.arange(len(r_s)) - np.searchsorted(r_s, r_s)
            g_of = r_s // 128
            lane = r_s % 128
            assert g_of.max(initial=0) < Gc
            assert (j < smax[g_of]).all()
            slot = (base[g_of] + j) * 128 + lane
            stream = np.full(totslots, ZREL, np.int64)
            stream[slot] = src_s
            per_core[k][f"idx{c}"] = _wrap16(stream)
            # scatter stream: group-major lanes -> local dest or trash row NLOC
            sidx = np.full(Gc * 128, NLOC, np.int64)
            nd = int((cnt > 0).sum())
            rr = np.arange(nd)
            sidx[rr] = orderd[:nd]
            per_core[k][f"sidx{c}"] = _wrap16(sidx)

    # ---- Set2Set structures ----
    gsizes = np.bincount(batch, minlength=B)
    PMAX = int(-(-(gsizes.max()) // 8) * 8)
    for k in range(NCORES):
        idx = np.full(128 * PMAX, NLOC, np.int64)   # zero row
        mask = np.full((128, PMAX), -1e9, np.float32)
        for gi in range(GPC):
            g = k * GPC + gi
            s0 = int(np.searchsorted(batch, g))
            sz = int(gsizes[g])
            sl_ = np.arange(sz)
            idx[sl_ * 128 + gi] = (s0 - nstart[k]) + sl_
            mask[gi, :sz] = 0.0
        per_core[k]["s2s_idx"] = _wrap16(idx)
        per_core[k]["s2s_mask"] = mask

    # ---- weights (shared) ----
    shared = {
        "emb": np.asarray(emb, np.float32),
        "convW": np.asarray(conv_W, np.float32),
        "convB": np.asarray(conv_b, np.float32).reshape(NCONVS, 1, F),
        "WihT": np.ascontiguousarray(np.asarray(lstm_Wih, np.float32).T),   # [256,512]
        "WhhT": np.ascontiguousarray(np.asarray(lstm_Whh, np.float32).T),   # [128,512]
        "bih": np.asarray(lstm_bih, np.float32).reshape(1, 4 * F),
        "bhh": np.asarray(lstm_bhh, np.float32).reshape(1, 4 * F),
        "W0T": np.ascontiguousarray(np.asarray(lin_W0, np.float32).T),      # [256,128]
        "b0": np.asarray(lin_b0, np.float32).reshape(1, F),
        "W1T": np.ascontiguousarray(np.asarray(lin_W1, np.float32).T),      # [128,64]
        "b1": np.asarray(lin_b1, np.float32).reshape(1, 64),
        "W3T": np.ascontiguousarray(np.asarray(lin3_W, np.float32).T),      # [64,1]
        "b3": np.asarray(lin3_b, np.float32).reshape(1, 1),
    }
    meta = dict(sched=sched, g_counts=g_counts, PMAX=PMAX)
    return per_core, shared, meta


# ---------------- device program ----------------

def _build(cfg, meta, trunc="full"):
    from concourse import bass, bacc, mybir, tile
    from concourse.masks import make_identity

    f16, f32, i16 = mybir.dt.float16, mybir.dt.float32, mybir.dt.int16
    NLOC, CHUNK, TROWS, G = cfg.NLOC, cfg.CHUNK, cfg.TROWS, cfg.G
    NSLAB, SPC, GPC = cfg.NSLAB, cfg.SLABS_PER_CHUNK, cfg.GPC
    sched, g_counts, PMAX = meta["sched"], meta["g_counts"], meta["PMAX"]
    AF = mybir.ActivationFunctionType
    ALU = mybir.AluOpType
    AX = mybir.AxisListType

    nc = bacc.Bacc("TRN2", target_bir_lowering=False, debug=False,
                   num_devices=NCORES)

    def din(name, shape, dt):
        return nc.dram_tensor(name, shape, dt, kind="ExternalInput").ap()

    # inputs
    x16 = din("x16", [128, NLOC // 16], i16)
    dinv_in = din("dinv", [128, G], f32)
    idx_in = [din(f"idx{c}", [128, meta_cols(cfg, sched, c)], i16) for c in range(4)]
    sidx_in = [din(f"sidx{c}", [128, g_counts[c] * 8], i16) for c in range(4)]
    s2s_idx_in = din("s2s_idx", [128, 128 * PMAX // 16], i16)
    s2s_mask_in = din("s2s_mask", [128, PMAX], f32)
    emb_in = din("emb", [NEMB, F], f32)
    convW_in = din("convW", [NCONVS, F, F], f32)
    convB_in = din("convB", [NCONVS, 1, F], f32)
    WihT_in = din("WihT", [2 * F, 4 * F], f32)
    WhhT_in = din("WhhT", [F, 4 * F], f32)
    bih_in = din("bih", [1, 4 * F], f32)
    bhh_in = din("bhh", [1, 4 * F], f32)
    W0T_in = din("W0T", [2 * F, F], f32)
    b0_in = din("b0", [1, F], f32)
    W1T_in = din("W1T", [F, 64], f32)
    b1_in = din("b1", [1, 64], f32)
    W3T_in = din("W3T", [64, 1], f32)
    b3_in = din("b3", [1, 1], f32)

    out = nc.dram_tensor("out", [GPC, 1], f32, kind="ExternalOutput").ap()

    # internal DRAM
    bounce = [nc.dram_tensor(f"bounce{c}", [CHUNK, F], f16).ap() for c in range(4)]
    table = [nc.dram_tensor(f"table{c}", [TROWS + 128, F], f16,
                            addr_space="Shared").ap() for c in range(4)]
    accs = [nc.dram_tensor(f"acc{i}", [NLOC + 128, F], f16).ap()
            for i in range(NCONVS)]
    h_dram = nc.dram_tensor("h_dram", [NLOC + 128, F], f32).ap()

    rg = [list(range(NCORES))]

    with tile.TileContext(nc) as tc, nc.allow_low_precision("fp16 partial aggregation by design"):
        with (
            tc.tile_pool(name="consts", bufs=1) as cn,
            tc.tile_pool(name="psum_w", bufs=1, space="PSUM") as psw,
        ):
            ident = cn.tile([128, 128], f32)
            make_identity(nc, ident[:])
            ones1 = cn.tile([1, 128], f32)
            nc.vector.memset(ones1[:], 1.0)
            zslab16 = cn.tile([128, 4 * F], f16)
            nc.vector.memset(zslab16[:], 0)

            # resident weight tiles
            dinv_sb = cn.tile([128, G], f32)
            nc.sync.dma_start(out=dinv_sb[:], in_=dinv_in[:, :])
            convW_sb = []
            for i in range(NCONVS):
                t = cn.tile([128, F], f32, tag=f"convW{i}")
                nc.sync.dma_start(out=t[:], in_=convW_in[i, :, :])
                convW_sb.append(t)

            # zero the accumulators, table zero-rows, h_dram pad rows
            for i in range(NCONVS):
                for t in range(NSLAB):
                    nc.sync.dma_start(
                        out=accs[i][t * 512:(t + 1) * 512, :]
                        .rearrange("(g p) f -> p g f", p=128),
                        in_=zslab16[:].rearrange("p (g f) -> p g f", f=F))
                nc.sync.dma_start(
                    out=accs[i][NLOC:NLOC + 128, :],
                    in_=zslab16[:, :F])
            for c in range(4):
                nc.sync.dma_start(out=table[c][TROWS:TROWS + 128, :],
                                  in_=zslab16[:, :F])
            zslab32 = cn.tile([128, F], f32)
            nc.vector.memset(zslab32[:], 0)
            nc.sync.dma_start(out=h_dram[NLOC:NLOC + 128, :], in_=zslab32[:])

            # per-conv bias broadcast tiles [128, F] via ones-matmul
            bias_bc = []
            for i in range(NCONVS):
                bsb = cn.tile([1, F], f32, tag=f"bsb{i}")
                nc.sync.dma_start(out=bsb[:], in_=convB_in[i, :, :])
                bps = psw.tile([128, F], f32, tag="biasps")
                nc.tensor.matmul(out=bps[:], lhsT=ones1[:], rhs=bsb[:],
                                 start=True, stop=True)
                bb = cn.tile([128, F], f32, tag=f"biasbc{i}")
                nc.vector.tensor_copy(out=bb[:], in_=bps[:])
                bias_bc.append(bb)

            def slab_emit_xw(slab_i, h_slab_ap, Wi_sb, work, psum):
                """h~ slab [128,4,F] f32 -> transpose -> matmul W -> fp16 -> bounce."""
                tp = psum.tile([128, 4 * F], f32, tag="tps")
                for g in range(4):
                    nc.tensor.transpose(
                        out=tp[:, g * F:(g + 1) * F],
                        in_=h_slab_ap[:, g, :], identity=ident[:])
                hT = work.tile([128, 4 * F], f32, tag="hT")
                nc.vector.tensor_copy(out=hT[:], in_=tp[:])
                mm = psum.tile([128, 4 * F], f32, tag="mmps")
                for g in range(4):
                    nc.tensor.matmul(out=mm[:, g * F:(g + 1) * F],
                                     lhsT=hT[:, g * F:(g + 1) * F],
                                     rhs=Wi_sb[:], start=True, stop=True)
                xw16 = work.tile([128, 4 * F], f16, tag="xw16")
                nc.vector.tensor_copy(out=xw16[:], in_=mm[:])
                c = slab_i // SPC
                r0 = (slab_i % SPC) * 512
                nc.sync.dma_start(
                    out=bounce[c][r0:r0 + 512, :].rearrange("(g p) f -> p g f", p=128),
                    in_=xw16[:].rearrange("p (g f) -> p g f", f=F))

            with (
                tc.tile_pool(name="idxp", bufs=1) as ixp,
                tc.tile_pool(name="work", bufs=3) as wk,
                tc.tile_pool(name="gath", bufs=3) as gp,
                tc.tile_pool(name="stage", bufs=2) as stp,
                tc.tile_pool(name="psum", bufs=2, space="PSUM") as ps,
            ):
                x16_sb = ixp.tile([128, NLOC // 16], i16)
                nc.sync.dma_start(out=x16_sb[:], in_=x16[:, :])
                idx_sb = []
                for c in range(4):
                    t = ixp.tile([128, idx_in[c].shape[1]], i16, tag=f"idxsb{c}")
                    nc.sync.dma_start(out=t[:], in_=idx_in[c][:, :])
                    idx_sb.append(t)
                sidx_sb = []
                for c in range(4):
                    t = ixp.tile([128, g_counts[c] * 8], i16, tag=f"sidxsb{c}")
                    nc.sync.dma_start(out=t[:], in_=sidx_in[c][:, :])
                    sidx_sb.append(t)
                # ---- embedding phase: h~0 slabs -> bounce (xw~ for conv 0) ----
                for t in range(NSLAB):
                    h0g = wk.tile([128, 4 * F], f32, tag="h0g")
                    nc.gpsimd.dma_gather(
                        h0g[:].rearrange("p (g f) -> p g f", f=F),
                        emb_in[:, :],
                        x16_sb[:, t * 32:(t + 1) * 32],
                        512, 512, F, single_packet=False)
                    hq = wk.tile([128, 4 * F], f32, tag="hq")
                    nc.vector.tensor_tensor(
                        out=hq[:].rearrange("p (g f) -> p g f", f=F),
                        in0=h0g[:].rearrange("p (g f) -> p g f", f=F),
                        in1=dinv_sb[:, t * 4:(t + 1) * 4]
                        .rearrange("p (g o) -> p g o", o=1)
                        .to_broadcast([128, 4, F]),
                        op=ALU.mult)
                    slab_emit_xw(t, hq[:].rearrange("p (g f) -> p g f", f=F),
                                 convW_sb[0], wk, ps)

                # ---- conv loop ----
                NCV = 0 if trunc == "h0" else (1 if trunc in ("ag0", "gather0", "conv0") else NCONVS)
                for i in range(NCV):
                    for c in range(4):
                        if trunc == "noag":
                            nc.gpsimd.dma_start(
                                out=table[c][0:TROWS, :], in_=bounce[c][:, :])
                        else:
                            nc.gpsimd.collective_compute(
                                "AllGather", ALU.bypass, replica_groups=rg,
                                ins=[bounce[c].opt()],
                                outs=[table[c][0:TROWS, :].opt()])
                    if trunc == "ag0":
                        break
                    for c in range(4):
                        Gc = g_counts[c]
                        stage = stp.tile([128, Gc * F], f16, tag="stage")
                        coloff = 0
                        for (g0, kk, S) in sched[c]:
                            nidx = kk * S * 128
                            gt = gp.tile([128, kk * S * F], f16, tag="gt")
                            nc.gpsimd.dma_gather(
                                gt[:].rearrange("p (n f) -> p n f", f=F),
                                table[c][:, :],
                                idx_sb[c][:, coloff:coloff + nidx // 16],
                                nidx, nidx, F, single_packet=False)
                            nc.vector.tensor_reduce(
                                out=stage[:, g0 * F:(g0 + kk) * F]
                                .rearrange("p (g f) -> p g f", f=F),
                                in_=gt[:].rearrange("p (g s f) -> p g f s", s=S, f=F),
                                axis=AX.X, op=ALU.add)
                            coloff += nidx // 16
                        if trunc != "nosc":
                            for s0 in range(0, Gc, 48):
                                sn = min(48, Gc - s0)
                                nc.gpsimd.dma_scatter_add(
                                    accs[i][:, :],
                                    stage[:, s0 * F:(s0 + sn) * F]
                                    .rearrange("p (g f) -> p g f", f=F),
                                    sidx_sb[c][:, s0 * 8:(s0 + sn) * 8],
                                    sn * 128, sn * 128, F, single_packet=False)
                    if trunc == "gather0":
                        break
                    # epilogue: acc -> h (relu(dinv*sum + b)) -> h~ -> next xw~
                    if trunc == "conv0" and i == 0:
                        pass
                    for t in range(NSLAB):
                        asl = wk.tile([128, 4 * F], f16, tag="asl")
                        nc.sync.dma_start(
                            out=asl[:].rearrange("p (g f) -> p g f", f=F),
                            in_=accs[i][t * 512:(t + 1) * 512, :]
                            .rearrange("(g p) f -> p g f", p=128))
                        dv = (dinv_sb[:, t * 4:(t + 1) * 4]
                              .rearrange("p (g o) -> p g o", o=1)
                              .to_broadcast([128, 4, F]))
                        u = wk.tile([128, 4 * F], f32, tag="u")
                        nc.vector.tensor_tensor(
                            out=u[:].rearrange("p (g f) -> p g f", f=F),
                            in0=asl[:].rearrange("p (g f) -> p g f", f=F),
                            in1=dv, op=ALU.mult)
                        nc.vector.tensor_tensor(
                            out=u[:].rearrange("p (g f) -> p g f", f=F),
                            in0=u[:].rearrange("p (g f) -> p g f", f=F),
                            in1=bias_bc[i][:].rearrange("p (o f) -> p o f", o=1)
                            .to_broadcast([128, 4, F]),
                            op=ALU.add)
                        h = wk.tile([128, 4 * F], f32, tag="h")
                        nc.scalar.activation(out=h[:], in_=u[:], func=AF.Relu)
                        if i == NCONVS - 1:
                            nc.sync.dma_start(
                                out=h_dram[t * 512:(t + 1) * 512, :]
                                .rearrange("(g p) f -> p g f", p=128),
                                in_=h[:].rearrange("p (g f) -> p g f", f=F))
                        else:
                            hq = wk.tile([128, 4 * F], f32, tag="hq")
                            nc.vector.tensor_tensor(
                                out=hq[:].rearrange("p (g f) -> p g f", f=F),
                                in0=h[:].rearrange("p (g f) -> p g f", f=F),
                                in1=dv, op=ALU.mult)
                            slab_emit_xw(t, hq[:].rearrange("p (g f) -> p g f", f=F),
                                         convW_sb[i + 1], wk, ps)

            # ---- Set2Set + head ----
            do_s2s = trunc not in ("h0", "ag0", "gather0", "conv0")
            if do_s2s:
              with (
                tc.tile_pool(name="s2s", bufs=1) as sp,
                tc.tile_pool(name="s2w", bufs=1) as sw,
                tc.tile_pool(name="ps2", bufs=1, space="PSUM") as ps2,
            ):
                s2s_idx_sb = sp.tile([128, 128 * PMAX // 16], i16)
                nc.sync.dma_start(out=s2s_idx_sb[:], in_=s2s_idx_in[:, :])
                mask_sb = sp.tile([128, PMAX], f32)
                nc.sync.dma_start(out=mask_sb[:], in_=s2s_mask_in[:, :])
                WihT_sb = sp.tile([128, 2 * 4 * F], f32)  # two K-chunks side by side
                nc.sync.dma_start(out=WihT_sb[:, :4 * F], in_=WihT_in[0:128, :])
                nc.sync.dma_start(out=WihT_sb[:, 4 * F:], in_=WihT_in[128:256, :])
                WhhT_sb = sp.tile([128, 4 * F], f32)
                nc.sync.dma_start(out=WhhT_sb[:], in_=WhhT_in[:, :])
                bsum = sp.tile([1, 4 * F], f32)
                bihs = sw.tile([1, 4 * F], f32, tag="bihs")
                nc.sync.dma_start(out=bihs[:], in_=bih_in[:, :])
                bhhs = sw.tile([1, 4 * F], f32, tag="bhhs")
                nc.sync.dma_start(out=bhhs[:], in_=bhh_in[:, :])
                nc.vector.tensor_tensor(out=bsum[:], in0=bihs[:], in1=bhhs[:],
                                        op=ALU.add)
                W0T_sb = sp.tile([128, 2 * F], f32)
                nc.sync.dma_start(out=W0T_sb[:, :F], in_=W0T_in[0:128, :])
                nc.sync.dma_start(out=W0T_sb[:, F:], in_=W0T_in[128:256, :])
                b0_sb = sp.tile([1, F], f32)
                nc.sync.dma_start(out=b0_sb[:], in_=b0_in[:, :])
                W1T_sb = sp.tile([128, 64], f32)
                nc.sync.dma_start(out=W1T_sb[:], in_=W1T_in[:, :])
                b1_sb = sp.tile([1, 64], f32)
                nc.sync.dma_start(out=b1_sb[:], in_=b1_in[:, :])
                W3T_sb = sp.tile([64, 1], f32)
                nc.sync.dma_start(out=W3T_sb[:], in_=W3T_in[:, :])
                b3_sb = sp.tile([1, 1], f32)
                nc.sync.dma_start(out=b3_sb[:], in_=b3_in[:, :])

                hs = sp.tile([128, PMAX * F], f32)     # [graph, slot, feat]
                for s0 in range(0, PMAX, 8):
                    nc.gpsimd.dma_gather(
                        hs[:].rearrange("p (s f) -> p s f", f=F)[:, s0:s0 + 8, :],
                        h_dram[:, :],
                        s2s_idx_sb[:, s0 * 8:(s0 + 8) * 8],
                        8 * 128, 8 * 128, F, single_packet=False)

                qs = sp.tile([128, 2 * F], f32)
                nc.vector.memset(qs[:], 0)
                hh = sp.tile([128, F], f32)
                nc.vector.memset(hh[:], 0)
                cc = sp.tile([128, F], f32)
                nc.vector.memset(cc[:], 0)
                SCH = 40

                def transpose_to(dst_sb, src_ap, width):
                    tp = ps2.tile([128, 128], f32, tag="tp2")
                    nc.tensor.transpose(out=tp[:width, :], in_=src_ap,
                                        identity=ident[:])
                    nc.vector.tensor_copy(out=dst_sb[:width, :], in_=tp[:width, :])

                for _step in range(STEPS):
                    qsT = sw.tile([128, 2 * 128], f32, tag="qsT")
                    transpose_to(qsT[:, 0:128], qs[:, 0:F], 128)
                    transpose_to(qsT[:, 128:256], qs[:, F:2 * F], 128)
                    hhT = sw.tile([128, 128], f32, tag="hhT")
                    transpose_to(hhT, hh[:], 128)
                    gates = ps2.tile([128, 4 * F], f32, tag="gates")
                    nc.tensor.matmul(out=gates[:], lhsT=qsT[:, 0:128],
                                     rhs=WihT_sb[:, :4 * F], start=True, stop=False)
                    nc.tensor.matmul(out=gates[:], lhsT=qsT[:, 128:256],
                                     rhs=WihT_sb[:, 4 * F:], start=False, stop=False)
                    nc.tensor.matmul(out=gates[:], lhsT=hhT[:],
                                     rhs=WhhT_sb[:], start=False, stop=False)
                    nc.tensor.matmul(out=gates[:], lhsT=ones1[:],
                                     rhs=bsum[:], start=False, stop=True)
                    ig = sw.tile([128, F], f32, tag="ig")
                    nc.scalar.activation(out=ig[:], in_=gates[:, 0:F], func=AF.Sigmoid)
                    fg = sw.tile([128, F], f32, tag="fg")
                    nc.scalar.activation(out=fg[:], in_=gates[:, F:2 * F], func=AF.Sigmoid)
                    gg = sw.tile([128, F], f32, tag="gg")
                    nc.scalar.activation(out=gg[:], in_=gates[:, 2 * F:3 * F], func=AF.Tanh)
                    og = sw.tile([128, F], f32, tag="og")
                    nc.scalar.activation(out=og[:], in_=gates[:, 3 * F:4 * F], func=AF.Sigmoid)
                    t1 = sw.tile([128, F], f32, tag="t1")
                    nc.vector.tensor_tensor(out=t1[:], in0=fg[:], in1=cc[:], op=ALU.mult)
                    t2 = sw.tile([128, F], f32, tag="t2")
                    nc.vector.tensor_tensor(out=t2[:], in0=ig[:], in1=gg[:], op=ALU.mult)
                    nc.vector.tensor_tensor(out=cc[:], in0=t1[:], in1=t2[:], op=ALU.add)
                    tnc = sw.tile([128, F], f32, tag="tnc")
                    nc.scalar.activation(out=tnc[:], in_=cc[:], func=AF.Tanh)
                    nc.vector.tensor_tensor(out=hh[:], in0=og[:], in1=tnc[:], op=ALU.mult)

                    # attention (slot-chunked to bound SBUF)
                    e = sw.tile([128, PMAX], f32, tag="e")
                    for c0 in range(0, PMAX, SCH):
                        cw = min(SCH, PMAX - c0)
                        prodc = sw.tile([128, SCH * F], f32, tag="prodc")
                        nc.vector.tensor_tensor(
                            out=prodc[:, :cw * F].rearrange("p (s f) -> p s f", f=F),
                            in0=hs[:].rearrange("p (s f) -> p s f", f=F)[:, c0:c0 + cw, :],
                            in1=hh[:].rearrange("p (o f) -> p o f", o=1)
                            .to_broadcast([128, cw, F]),
                            op=ALU.mult)
                        nc.vector.tensor_reduce(
                            out=e[:, c0:c0 + cw],
                            in_=prodc[:, :cw * F].rearrange("p (s f) -> p s f", f=F),
                            axis=AX.X, op=ALU.add)
                    nc.vector.tensor_tensor(out=e[:], in0=e[:], in1=mask_sb[:],
                                            op=ALU.add)
                    negm = sw.tile([128, 1], f32, tag="negm")
                    nc.vector.tensor_reduce(out=negm[:], in_=e[:], axis=AX.X,
                                            op=ALU.max, negate=True)
                    ex = sw.tile([128, PMAX], f32, tag="ex")
                    nc.scalar.activation(out=ex[:], in_=e[:], func=AF.Exp,
                                         bias=negm[:, :], scale=1.0)
                    ssum = sw.tile([128, 1], f32, tag="ssum")
                    nc.vector.tensor_reduce(out=ssum[:], in_=ex[:], axis=AX.X,
                                            op=ALU.add)
                    rinv = sw.tile([128, 1], f32, tag="rinv")
                    nc.vector.reciprocal(out=rinv[:], in_=ssum[:])
                    a = sw.tile([128, PMAX], f32, tag="a")
                    nc.vector.tensor_tensor(out=a[:], in0=ex[:],
                                            in1=rinv[:].to_broadcast([128, PMAX]),
                                            op=ALU.mult)
                    r = sw.tile([128, F], f32, tag="r")
                    nc.vector.memset(r[:], 0)
                    for c0 in range(0, PMAX, SCH):
                        cw = min(SCH, PMAX - c0)
                        prodc = sw.tile([128, SCH * F], f32, tag="prodc")
                        nc.vector.tensor_tensor(
                            out=prodc[:, :cw * F].rearrange("p (s f) -> p s f", f=F),
                            in0=hs[:].rearrange("p (s f) -> p s f", f=F)[:, c0:c0 + cw, :],
                            in1=a[:, c0:c0 + cw].rearrange("p (s o) -> p s o", o=1)
                            .to_broadcast([128, cw, F]),
                            op=ALU.mult)
                        rq = sw.tile([128, F], f32, tag="rq")
                        nc.vector.tensor_reduce(
                            out=rq[:],
                            in_=prodc[:, :cw * F].rearrange("p (s f) -> p f s", f=F),
                            axis=AX.X, op=ALU.add)
                        nc.vector.tensor_tensor(out=r[:], in0=r[:], in1=rq[:],
                                                op=ALU.add)
                    nc.vector.tensor_copy(out=qs[:, 0:F], in_=hh[:])
                    nc.vector.tensor_copy(out=qs[:, F:2 * F], in_=r[:])

                # MLP head
                qsT = sw.tile([128, 2 * 128], f32, tag="qsT")
                transpose_to(qsT[:, 0:128], qs[:, 0:F], 128)
                transpose_to(qsT[:, 128:256], qs[:, F:2 * F], 128)
                z1p = ps2.tile([128, F], f32, tag="z1p")
                nc.tensor.matmul(out=z1p[:], lhsT=qsT[:, 0:128],
                                 rhs=W0T_sb[:, :F], start=True, stop=False)
                nc.tensor.matmul(out=z1p[:], lhsT=qsT[:, 128:256],
                                 rhs=W0T_sb[:, F:], start=False, stop=False)
                nc.tensor.matmul(out=z1p[:], lhsT=ones1[:], rhs=b0_sb[:],
                                 start=False, stop=True)
                z1 = sw.tile([128, F], f32, tag="z1")
                nc.scalar.activation(out=z1[:], in_=z1p[:], func=AF.Relu)
                z1T = sw.tile([128, 128], f32, tag="z1T")
                transpose_to(z1T, z1[:], 128)
                z2p = ps2.tile([128, 64], f32, tag="z2p")
                nc.tensor.matmul(out=z2p[:], lhsT=z1T[:], rhs=W1T_sb[:],
                                 start=True, stop=False)
                nc.tensor.matmul(out=z2p[:], lhsT=ones1[:], rhs=b1_sb[:],
                                 start=False, stop=True)
                z2 = sw.tile([128, 64], f32, tag="z2")
                nc.scalar.activation(out=z2[:], in_=z2p[:], func=AF.Relu)
                z2T = sw.tile([64, 128], f32, tag="z2T")
                tp = ps2.tile([128, 128], f32, tag="tp3")
                nc.tensor.transpose(out=tp[:64, :], in_=z2[:], identity=ident[:])
                nc.vector.tensor_copy(out=z2T[:, :], in_=tp[:64, :])
                z3p = ps2.tile([128, 1], f32, tag="z3p")
                nc.tensor.matmul(out=z3p[:], lhsT=z2T[:, :], rhs=W3T_sb[:],
                                 start=True, stop=False)
                nc.tensor.matmul(out=z3p[:], lhsT=ones1[:], rhs=b3_sb[:],
                                 start=False, stop=True)
                z3 = sw.tile([128, 1], f32, tag="z3")
                nc.vector.tensor_copy(out=z3[:], in_=z3p[:])
                nc.sync.dma_start(out=out[:, :], in_=z3[:GPC, :])

    nc.compile()
    return nc


def meta_cols(cfg, sched, c):
    tot = sum(kk * S for (_, kk, S) in sched[c]) * 128
    return tot // 16


# ---------------- runner ----------------

def _run(cfg, inputs, use_sim=False, trace=False):
    global LAST_EXEC_NS
    per_core, shared, meta = _prep(cfg, **inputs)
    nc = _build(cfg, meta, trunc=os.environ.get("GCN_TRUNC", "full"))
    in_maps = []
    for k in range(NCORES):
        m = dict(shared)
        m.update(per_core[k])
        m = {name: np.ascontiguousarray(v) for name, v in m.items()}
        in_maps.append(m)
    if use_sim:
        from concourse import bass_interp
        sim = bass_interp.MultiCoreSim(nc, NCORES)
        for k in range(NCORES):
            for name, v in in_maps[k].items():
                sim.cores[k].tensor(name)[:] = v
        sim.simulate(check_with_hw=False)
        outs = [np.array(sim.cores[k].mem_tensor("out")) for k in range(NCORES)]
    else:
        from concourse.bass_utils import run_bass_kernel_spmd
        if trace:
            _install_ntff_hook()
        res = run_bass_kernel_spmd(nc, in_maps, core_ids=list(range(NCORES)),
                                   trace=trace)
        LAST_EXEC_NS = res.exec_time_ns
        outs = [res.results[k]["out"] for k in range(NCORES)]
    return np.concatenate(outs, axis=0).astype(np.float32)


def kernel(**inputs) -> np.ndarray:
    trace = bool(os.environ.get("GCN_TRACE"))
    return _run(FULL, inputs, use_sim=False, trace=trace)


# revision 10
# speedup vs baseline: 1.0473x; 1.0473x over previous
"""Self-contained Bass/Trainium2 kernel for nn_BasicGCN (4x GCNConv + Set2Set + MLP).

Distribution: nodes sharded by graph across 8 cores (graph-level data parallelism).
Per conv: local xw~ = (dinv*h) @ W computed on PE, fp16-AllGathered in 4 row-chunks
(each chunk tensor = one int16-addressable dma_gather window), per-edge gather via
dma_gather into degree-sorted padded [128-dest, slots, F] tiles, DVE slot-reduce,
dma_scatter_add combine of the 4 chunk-partials into an fp16 accumulator, then
per-partition dinv scale + bias + relu epilogue. Set2Set runs per-graph with graphs
on partitions (uniform padded slot layout); LSTM + MLP head on PE/ACT/DVE.
"""
import os
import numpy as np

# ---------------- configuration ----------------
F = 128          # feature dim (H == EMB)
NCORES = 8
NCONVS = 4
STEPS = 2
NEMB = 118


class Cfg:
    def __init__(self, N, E, B, NLOC, slot_cap=48):
        assert NLOC % 2048 == 0
        self.N, self.E, self.B = N, E, B
        self.NLOC = NLOC                 # padded local node rows (mult of 2048)
        self.CHUNK = NLOC // 4           # local rows per AG chunk
        self.TROWS = 8 * self.CHUNK      # data rows per chunk table
        assert self.TROWS + 128 <= 32768  # int16 gather index limit
        self.GPC = B // NCORES           # graphs per core
        self.G = NLOC // 128             # dest groups per core
        self.NSLAB = NLOC // 512         # epilogue slabs (4 groups each)
        self.SLABS_PER_CHUNK = self.CHUNK // 512
        self.slot_cap = slot_cap


FULL = Cfg(N=100000, E=1600000, B=1000, NLOC=14336)

LAST_EXEC_NS = None


def _install_ntff_hook():
    """Best-effort: make run_bass_kernel_spmd(trace=True) work under axon."""
    import contextlib, ctypes, sys, types
    try:
        import antenv.axon_hooks  # noqa: F401
        return
    except ImportError:
        pass
    cands = []
    try:
        for line in open("/proc/self/maps"):
            if "libaxon_pjrt.so" in line:
                cands.append(line.split()[-1])
                break
    except OSError:
        pass
    cands.append("/opt/axon/libaxon_pjrt.so")
    lib = None
    for so_path in cands:
        if not os.path.exists(so_path):
            continue
        try:
            cand = ctypes.CDLL(so_path)
        except OSError:
            continue
        if hasattr(cand, "axon_start_nrt_profile"):
            lib = cand
            break
    if lib is None:
        return
    lib.axon_start_nrt_profile.argtypes = [ctypes.POINTER(ctypes.c_int64), ctypes.c_size_t]
    lib.axon_start_nrt_profile.restype = ctypes.c_int64
    lib.axon_stop_nrt_profile.argtypes = [ctypes.c_char_p]
    lib.axon_stop_nrt_profile.restype = ctypes.c_int64

    @contextlib.contextmanager
    def _hook(output_dir, device_ids):
        import jax
        jax.devices()
        if device_ids:
            ids = (ctypes.c_int64 * len(device_ids))(*device_ids)
            rc = lib.axon_start_nrt_profile(ids, len(device_ids))
        else:
            rc = lib.axon_start_nrt_profile(None, 0)
        if rc != 0:
            raise RuntimeError(f"axon_start_nrt_profile rc={rc}")
        try:
            yield
        finally:
            lib.axon_stop_nrt_profile(str(output_dir).encode())

    mod = types.ModuleType("antenv.axon_hooks")
    mod.get_axon_ntff_profile_hook = lambda: _hook
    mod.set_axon_ntff_profile_hook = lambda h: None
    sys.modules["antenv.axon_hooks"] = mod
    try:
        import antenv
        antenv.axon_hooks = mod
    except ImportError:
        pass
    try:
        from concourse import bass_utils
        bass_utils.upload_artifacts = lambda tmpdir: f"local://{tmpdir}"
    except ImportError:
        pass


# ---------------- host-side preprocessing ----------------

def _wrap16(a):
    """int array [n] (n % 16 == 0) -> int16 [128, n/16] wrapped+replicated layout."""
    a = np.asarray(a)
    n = a.shape[0]
    assert n % 16 == 0
    assert a.min() >= 0 and a.max() < 32768, (a.min(), a.max())
    return np.tile(a.reshape(n // 16, 16).T, (8, 1)).astype(np.int16)


def _prep(cfg, x, edge_index, batch, emb, conv_W, conv_b,
          lstm_Wih, lstm_Whh, lstm_bih, lstm_bhh,
          lin_W0, lin_b0, lin_W1, lin_b1, lin3_W, lin3_b):
    N, B, NLOC, CHUNK, GPC = cfg.N, cfg.B, cfg.NLOC, cfg.CHUNK, cfg.GPC
    x = np.asarray(x).astype(np.int64)
    batch = np.asarray(batch).astype(np.int64)
    row = np.asarray(edge_index[0]).astype(np.int64)
    col = np.asarray(edge_index[1]).astype(np.int64)

    # node -> core via graph id (batch is sorted)
    nstart = np.searchsorted(batch, np.arange(NCORES) * GPC)
    nstart = np.concatenate([nstart, [N]])
    nloc = np.diff(nstart)
    assert nloc.max() <= NLOC, (nloc.max(), NLOC)
    kof = np.repeat(np.arange(NCORES), nloc)       # core of node
    loc = np.arange(N) - nstart[kof]               # local index of node

    deg = np.bincount(col, minlength=N).astype(np.float64) + 1.0
    dinv = (deg ** -0.5).astype(np.float32)

    # ---- conv gather structures ----
    # real edges only; the self-loop term is added densely in the epilogue
    e_dst = col
    e_src = row
    kd = kof[e_dst]
    dl = loc[e_dst].astype(np.int64)
    sc = kof[e_src]
    sl = loc[e_src].astype(np.int64)
    ch = sl // CHUNK                               # AG chunk of the source
    srel = sc * CHUNK + (sl - ch * CHUNK)          # row within chunk table
    ZREL = cfg.TROWS                               # zero row in each chunk table

    # per (core, chunk) slot counts per local dest
    cnts = np.zeros((NCORES, 4, NLOC), np.int64)
    np.add.at(cnts, (kd, ch, dl), 1)

    # common schedule: per chunk, group count and per-group slot counts
    sched = []        # sched[c] = list of (g_start, k, S) gather batches
    g_counts = []     # g_counts[c] = number of 128-dest groups (common)
    s_per_group = []  # s_per_group[c] = np.array [G_c] of slot counts
    for c in range(4):
        percore_sorted = [np.sort(cnts[k, c])[::-1] for k in range(NCORES)]
        ndst = max(int((s > 0).sum()) for s in percore_sorted)
        Gc = max(1, -(-ndst // 128))
        smax = np.zeros(Gc, np.int64)
        for k in range(NCORES):
            s = percore_sorted[k]
            v = s[np.arange(Gc) * 128]             # group max = first of each 128-run
            smax = np.maximum(smax, v)
        smax = np.maximum(smax, 1)
        g_counts.append(Gc)
        s_per_group.append(smax)
        # batches: consecutive equal-S groups, k*S <= slot_cap
        batches = []
        g = 0
        while g < Gc:
            S = int(smax[g])
            k = 1
            while (g + k < Gc and int(smax[g + k]) == S
                   and (k + 1) * S <= max(cfg.slot_cap, S)):
                k += 1
            batches.append((g, k, S))
            g += k
        sched.append(batches)

    # per-core idx / scatter streams
    per_core = []
    order_e = np.lexsort((e_src, dl, kd))  # stable grouping helper (not strictly needed)
    del order_e
    for k in range(NCORES):
        core = {}
        # x + dinv
        xl = np.zeros(NLOC, np.int64)
        xl[: nloc[k]] = x[nstart[k]: nstart[k + 1]]
        core["x16"] = _wrap16(xl)
        dv = np.zeros(NLOC, np.float32)
        dv[: nloc[k]] = dinv[nstart[k]: nstart[k + 1]]
        core["dinv"] = np.ascontiguousarray(dv.reshape(cfg.G, 128).T)  # [128, G]
        per_core.append(core)

    for c in range(4):
        Gc = g_counts[c]
        smax = s_per_group[c]
        base = np.concatenate([[0], np.cumsum(smax)])
        totslots = int(base[-1]) * 128
        for k in range(NCORES):
            m = (kd == k) & (ch == c)
            dsts = dl[m]
            srcs = srel[m]
            cnt = cnts[k, c]
            # rank dests by (count desc, dest asc)
            orderd = np.lexsort((np.arange(NLOC), -cnt))
            rank_of = np.empty(NLOC, np.int64)
            rank_of[orderd] = np.arange(NLOC)
            r = rank_of[dsts]
            # position of each slot within its dest
            o = np.lexsort((srcs, r))
            r_s, src_s = r[o], srcs[o]
            j = np.arange(len(r_s)) - np.searchsorted(r_s, r_s)
            g_of = r_s // 128
            lane = r_s % 128
            assert g_of.max(initial=0) < Gc
            assert (j < smax[g_of]).all()
            slot = (base[g_of] + j) * 128 + lane
            stream = np.full(totslots, ZREL, np.int64)
            stream[slot] = src_s
            per_core[k][f"idx{c}"] = _wrap16(stream)
            # scatter stream: group-major lanes -> local dest or trash row NLOC
            sidx = np.full(Gc * 128, NLOC, np.int64)
            nd = int((cnt > 0).sum())
            rr = np.arange(nd)
            sidx[rr] = orderd[:nd]
            per_core[k][f"sidx{c}"] = _wrap16(sidx)

    # ---- Set2Set structures ----
    gsizes = np.bincount(batch, minlength=B)
    PMAX = int(-(-(gsizes.max()) // 8) * 8)
    for k in range(NCORES):
        idx = np.full(128 * PMAX, NLOC, np.int64)   # zero row
        mask = np.full((128, PMAX), -1e9, np.float32)
        for gi in range(GPC):
            g = k * GPC + gi
            s0 = int(np.searchsorted(batch, g))
            sz = int(gsizes[g])
            sl_ = np.arange(sz)
            idx[sl_ * 128 + gi] = (s0 - nstart[k]) + sl_
            mask[gi, :sz] = 0.0
        per_core[k]["s2s_idx"] = _wrap16(idx)
        per_core[k]["s2s_mask"] = mask

    # ---- weights (shared) ----
    shared = {
        "emb": np.asarray(emb, np.float32),
        "convW": np.asarray(conv_W, np.float32),
        "convB": np.asarray(conv_b, np.float32).reshape(NCONVS, 1, F),
        "WihT": np.ascontiguousarray(np.asarray(lstm_Wih, np.float32).T),   # [256,512]
        "WhhT": np.ascontiguousarray(np.asarray(lstm_Whh, np.float32).T),   # [128,512]
        "bih": np.asarray(lstm_bih, np.float32).reshape(1, 4 * F),
        "bhh": np.asarray(lstm_bhh, np.float32).reshape(1, 4 * F),
        "W0T": np.ascontiguousarray(np.asarray(lin_W0, np.float32).T),      # [256,128]
        "b0": np.asarray(lin_b0, np.float32).reshape(1, F),
        "W1T": np.ascontiguousarray(np.asarray(lin_W1, np.float32).T),      # [128,64]
        "b1": np.asarray(lin_b1, np.float32).reshape(1, 64),
        "W3T": np.ascontiguousarray(np.asarray(lin3_W, np.float32).T),      # [64,1]
        "b3": np.asarray(lin3_b, np.float32).reshape(1, 1),
    }
    meta = dict(sched=sched, g_counts=g_counts, PMAX=PMAX)
    return per_core, shared, meta


# ---------------- device program ----------------

def _build(cfg, meta, trunc="full"):
    from concourse import bass, bacc, mybir, tile
    from concourse.masks import make_identity

    f16, f32, i16 = mybir.dt.float16, mybir.dt.float32, mybir.dt.int16
    NLOC, CHUNK, TROWS, G = cfg.NLOC, cfg.CHUNK, cfg.TROWS, cfg.G
    NSLAB, SPC, GPC = cfg.NSLAB, cfg.SLABS_PER_CHUNK, cfg.GPC
    sched, g_counts, PMAX = meta["sched"], meta["g_counts"], meta["PMAX"]
    AF = mybir.ActivationFunctionType
    ALU = mybir.AluOpType
    AX = mybir.AxisListType

    nc = bacc.Bacc("TRN2", target_bir_lowering=False, debug=False,
                   num_devices=NCORES)

    def din(name, shape, dt):
        return nc.dram_tensor(name, shape, dt, kind="ExternalInput").ap()

    # inputs
    x16 = din("x16", [128, NLOC // 16], i16)
    dinv_in = din("dinv", [128, G], f32)
    idx_in = [din(f"idx{c}", [128, meta_cols(cfg, sched, c)], i16) for c in range(4)]
    sidx_in = [din(f"sidx{c}", [128, g_counts[c] * 8], i16) for c in range(4)]
    s2s_idx_in = din("s2s_idx", [128, 128 * PMAX // 16], i16)
    s2s_mask_in = din("s2s_mask", [128, PMAX], f32)
    emb_in = din("emb", [NEMB, F], f32)
    convW_in = din("convW", [NCONVS, F, F], f32)
    convB_in = din("convB", [NCONVS, 1, F], f32)
    WihT_in = din("WihT", [2 * F, 4 * F], f32)
    WhhT_in = din("WhhT", [F, 4 * F], f32)
    bih_in = din("bih", [1, 4 * F], f32)
    bhh_in = din("bhh", [1, 4 * F], f32)
    W0T_in = din("W0T", [2 * F, F], f32)
    b0_in = din("b0", [1, F], f32)
    W1T_in = din("W1T", [F, 64], f32)
    b1_in = din("b1", [1, 64], f32)
    W3T_in = din("W3T", [64, 1], f32)
    b3_in = din("b3", [1, 1], f32)

    out = nc.dram_tensor("out", [GPC, 1], f32, kind="ExternalOutput").ap()

    # internal DRAM
    bounce = [nc.dram_tensor(f"bounce{c}", [CHUNK, F], f16).ap() for c in range(4)]
    table = [nc.dram_tensor(f"table{c}", [TROWS + 128, F], f16,
                            addr_space="Shared").ap() for c in range(4)]
    accs = [nc.dram_tensor(f"acc{i}", [NLOC + 128, F], f16).ap()
            for i in range(NCONVS)]
    h_dram = nc.dram_tensor("h_dram", [NLOC + 128, F], f32).ap()

    rg = [list(range(NCORES))]

    with tile.TileContext(nc) as tc, nc.allow_low_precision("fp16 partial aggregation by design"):
        with (
            tc.tile_pool(name="consts", bufs=1) as cn,
            tc.tile_pool(name="psum_w", bufs=1, space="PSUM") as psw,
        ):
            ident = cn.tile([128, 128], f32)
            make_identity(nc, ident[:])
            ones1 = cn.tile([1, 128], f32)
            nc.vector.memset(ones1[:], 1.0)
            zslab16 = cn.tile([128, 4 * F], f16)
            nc.vector.memset(zslab16[:], 0)

            # resident weight tiles
            dinv_sb = cn.tile([128, G], f32)
            nc.sync.dma_start(out=dinv_sb[:], in_=dinv_in[:, :])
            convW_sb = []
            for i in range(NCONVS):
                t = cn.tile([128, F], f32, tag=f"convW{i}")
                nc.sync.dma_start(out=t[:], in_=convW_in[i, :, :])
                convW_sb.append(t)

            # zero the accumulators, table zero-rows, h_dram pad rows
            for i in range(NCONVS):
                for t in range(NSLAB):
                    nc.sync.dma_start(
                        out=accs[i][t * 512:(t + 1) * 512, :]
                        .rearrange("(g p) f -> p g f", p=128),
                        in_=zslab16[:].rearrange("p (g f) -> p g f", f=F))
                nc.sync.dma_start(
                    out=accs[i][NLOC:NLOC + 128, :],
                    in_=zslab16[:, :F])
            for c in range(4):
                nc.sync.dma_start(out=table[c][TROWS:TROWS + 128, :],
                                  in_=zslab16[:, :F])
            zslab32 = cn.tile([128, F], f32)
            nc.vector.memset(zslab32[:], 0)
            nc.sync.dma_start(out=h_dram[NLOC:NLOC + 128, :], in_=zslab32[:])

            # per-conv bias broadcast tiles [128, F] via ones-matmul
            bias_bc = []
            for i in range(NCONVS):
                bsb = cn.tile([1, F], f32, tag=f"bsb{i}")
                nc.sync.dma_start(out=bsb[:], in_=convB_in[i, :, :])
                bps = psw.tile([128, F], f32, tag="biasps")
                nc.tensor.matmul(out=bps[:], lhsT=ones1[:], rhs=bsb[:],
                                 start=True, stop=True)
                bb = cn.tile([128, F], f32, tag=f"biasbc{i}")
                nc.vector.tensor_copy(out=bb[:], in_=bps[:])
                bias_bc.append(bb)

            def slab_emit_xw(slab_i, h_slab_ap, Wi_sb, work, psum):
                """h~ slab [128,4,F] f32 -> transpose -> matmul W -> fp16 -> bounce."""
                tp = psum.tile([128, 4 * F], f32, tag="tps")
                for g in range(4):
                    nc.tensor.transpose(
                        out=tp[:, g * F:(g + 1) * F],
                        in_=h_slab_ap[:, g, :], identity=ident[:])
                hT = work.tile([128, 4 * F], f32, tag="hT")
                nc.vector.tensor_copy(out=hT[:], in_=tp[:])
                mm = psum.tile([128, 4 * F], f32, tag="mmps")
                for g in range(4):
                    nc.tensor.matmul(out=mm[:, g * F:(g + 1) * F],
                                     lhsT=hT[:, g * F:(g + 1) * F],
                                     rhs=Wi_sb[:], start=True, stop=True)
                xw16 = work.tile([128, 4 * F], f16, tag="xw16")
                nc.vector.tensor_copy(out=xw16[:], in_=mm[:])
                c = slab_i // SPC
                r0 = (slab_i % SPC) * 512
                nc.sync.dma_start(
                    out=bounce[c][r0:r0 + 512, :].rearrange("(g p) f -> p g f", p=128),
                    in_=xw16[:].rearrange("p (g f) -> p g f", f=F))

            with (
                tc.tile_pool(name="idxp", bufs=1) as ixp,
                tc.tile_pool(name="work", bufs=3) as wk,
                tc.tile_pool(name="gath", bufs=3) as gp,
                tc.tile_pool(name="stage", bufs=2) as stp,
                tc.tile_pool(name="psum", bufs=2, space="PSUM") as ps,
            ):
                x16_sb = ixp.tile([128, NLOC // 16], i16)
                nc.sync.dma_start(out=x16_sb[:], in_=x16[:, :])
                idx_sb = []
                for c in range(4):
                    t = ixp.tile([128, idx_in[c].shape[1]], i16, tag=f"idxsb{c}")
                    nc.sync.dma_start(out=t[:], in_=idx_in[c][:, :])
                    idx_sb.append(t)
                sidx_sb = []
                for c in range(4):
                    t = ixp.tile([128, g_counts[c] * 8], i16, tag=f"sidxsb{c}")
                    nc.sync.dma_start(out=t[:], in_=sidx_in[c][:, :])
                    sidx_sb.append(t)
                # ---- embedding phase: h~0 slabs -> bounce (xw~ for conv 0) ----
                for t in range(NSLAB):
                    h0g = wk.tile([128, 4 * F], f32, tag="h0g")
                    nc.gpsimd.dma_gather(
                        h0g[:].rearrange("p (g f) -> p g f", f=F),
                        emb_in[:, :],
                        x16_sb[:, t * 32:(t + 1) * 32],
                        512, 512, F, single_packet=False)
                    hq = wk.tile([128, 4 * F], f32, tag="hq")
                    nc.vector.tensor_tensor(
                        out=hq[:].rearrange("p (g f) -> p g f", f=F),
                        in0=h0g[:].rearrange("p (g f) -> p g f", f=F),
                        in1=dinv_sb[:, t * 4:(t + 1) * 4]
                        .rearrange("p (g o) -> p g o", o=1)
                        .to_broadcast([128, 4, F]),
                        op=ALU.mult)
                    slab_emit_xw(t, hq[:].rearrange("p (g f) -> p g f", f=F),
                                 convW_sb[0], wk, ps)

                # ---- conv loop ----
                NCV = 0 if trunc == "h0" else (1 if trunc in ("ag0", "gather0", "conv0") else NCONVS)
                for i in range(NCV):
                    for c in range(4):
                        if trunc == "noag":
                            nc.gpsimd.dma_start(
                                out=table[c][0:TROWS, :], in_=bounce[c][:, :])
                        else:
                            nc.gpsimd.collective_compute(
                                "AllGather", ALU.bypass, replica_groups=rg,
                                ins=[bounce[c].opt()],
                                outs=[table[c][0:TROWS, :].opt()])
                    if trunc == "ag0":
                        break
                    for c in range(4):
                        Gc = g_counts[c]
                        stage = stp.tile([128, Gc * F], f16, tag="stage")
                        coloff = 0
                        for (g0, kk, S) in sched[c]:
                            nidx = kk * S * 128
                            gt = gp.tile([128, kk * S * F], f16, tag="gt")
                            nc.gpsimd.dma_gather(
                                gt[:].rearrange("p (n f) -> p n f", f=F),
                                table[c][:, :],
                                idx_sb[c][:, coloff:coloff + nidx // 16],
                                nidx, nidx, F, single_packet=False)
                            nc.vector.tensor_reduce(
                                out=stage[:, g0 * F:(g0 + kk) * F]
                                .rearrange("p (g f) -> p g f", f=F),
                                in_=gt[:].rearrange("p (g s f) -> p g f s", s=S, f=F),
                                axis=AX.X, op=ALU.add)
                            coloff += nidx // 16
                        if trunc != "nosc":
                            for s0 in range(0, Gc, 48):
                                sn = min(48, Gc - s0)
                                nc.gpsimd.dma_scatter_add(
                                    accs[i][:, :],
                                    stage[:, s0 * F:(s0 + sn) * F]
                                    .rearrange("p (g f) -> p g f", f=F),
                                    sidx_sb[c][:, s0 * 8:(s0 + sn) * 8],
                                    sn * 128, sn * 128, F, single_packet=False)
                    if trunc == "gather0":
                        break
                    # epilogue: acc -> h (relu(dinv*sum + b)) -> h~ -> next xw~
                    if trunc == "conv0" and i == 0:
                        pass
                    for t in range(NSLAB):
                        asl = wk.tile([128, 4 * F], f16, tag="asl")
                        nc.sync.dma_start(
                            out=asl[:].rearrange("p (g f) -> p g f", f=F),
                            in_=accs[i][t * 512:(t + 1) * 512, :]
                            .rearrange("(g p) f -> p g f", p=128))
                        dv = (dinv_sb[:, t * 4:(t + 1) * 4]
                              .rearrange("p (g o) -> p g o", o=1)
                              .to_broadcast([128, 4, F]))
                        own = wk.tile([128, 4 * F], f16, tag="own")
                        cb = t // SPC
                        rb = (t % SPC) * 512
                        nc.sync.dma_start(
                            out=own[:].rearrange("p (g f) -> p g f", f=F),
                            in_=bounce[cb][rb:rb + 512, :]
                            .rearrange("(g p) f -> p g f", p=128))
                        u = wk.tile([128, 4 * F], f32, tag="u")
                        nc.vector.tensor_tensor(
                            out=u[:].rearrange("p (g f) -> p g f", f=F),
                            in0=asl[:].rearrange("p (g f) -> p g f", f=F),
                            in1=own[:].rearrange("p (g f) -> p g f", f=F),
                            op=ALU.add)
                        nc.vector.tensor_tensor(
                            out=u[:].rearrange("p (g f) -> p g f", f=F),
                            in0=u[:].rearrange("p (g f) -> p g f", f=F),
                            in1=dv, op=ALU.mult)
                        nc.vector.tensor_tensor(
                            out=u[:].rearrange("p (g f) -> p g f", f=F),
                            in0=u[:].rearrange("p (g f) -> p g f", f=F),
                            in1=bias_bc[i][:].rearrange("p (o f) -> p o f", o=1)
                            .to_broadcast([128, 4, F]),
                            op=ALU.add)
                        h = wk.tile([128, 4 * F], f32, tag="h")
                        nc.scalar.activation(out=h[:], in_=u[:], func=AF.Relu)
                        if i == NCONVS - 1:
                            nc.sync.dma_start(
                                out=h_dram[t * 512:(t + 1) * 512, :]
                                .rearrange("(g p) f -> p g f", p=128),
                                in_=h[:].rearrange("p (g f) -> p g f", f=F))
                        else:
                            hq = wk.tile([128, 4 * F], f32, tag="hq")
                            nc.vector.tensor_tensor(
                                out=hq[:].rearrange("p (g f) -> p g f", f=F),
                                in0=h[:].rearrange("p (g f) -> p g f", f=F),
                                in1=dv, op=ALU.mult)
                            slab_emit_xw(t, hq[:].rearrange("p (g f) -> p g f", f=F),
                                         convW_sb[i + 1], wk, ps)

            # ---- Set2Set + head ----
            do_s2s = trunc not in ("h0", "ag0", "gather0", "conv0")
            if do_s2s:
              with (
                tc.tile_pool(name="s2s", bufs=1) as sp,
                tc.tile_pool(name="s2w", bufs=1) as sw,
                tc.tile_pool(name="ps2", bufs=1, space="PSUM") as ps2,
            ):
                s2s_idx_sb = sp.tile([128, 128 * PMAX // 16], i16)
                nc.sync.dma_start(out=s2s_idx_sb[:], in_=s2s_idx_in[:, :])
                mask_sb = sp.tile([128, PMAX], f32)
                nc.sync.dma_start(out=mask_sb[:], in_=s2s_mask_in[:, :])
                WihT_sb = sp.tile([128, 2 * 4 * F], f32)  # two K-chunks side by side
                nc.sync.dma_start(out=WihT_sb[:, :4 * F], in_=WihT_in[0:128, :])
                nc.sync.dma_start(out=WihT_sb[:, 4 * F:], in_=WihT_in[128:256, :])
                WhhT_sb = sp.tile([128, 4 * F], f32)
                nc.sync.dma_start(out=WhhT_sb[:], in_=WhhT_in[:, :])
                bsum = sp.tile([1, 4 * F], f32)
                bihs = sw.tile([1, 4 * F], f32, tag="bihs")
                nc.sync.dma_start(out=bihs[:], in_=bih_in[:, :])
                bhhs = sw.tile([1, 4 * F], f32, tag="bhhs")
                nc.sync.dma_start(out=bhhs[:], in_=bhh_in[:, :])
                nc.vector.tensor_tensor(out=bsum[:], in0=bihs[:], in1=bhhs[:],
                                        op=ALU.add)
                W0T_sb = sp.tile([128, 2 * F], f32)
                nc.sync.dma_start(out=W0T_sb[:, :F], in_=W0T_in[0:128, :])
                nc.sync.dma_start(out=W0T_sb[:, F:], in_=W0T_in[128:256, :])
                b0_sb = sp.tile([1, F], f32)
                nc.sync.dma_start(out=b0_sb[:], in_=b0_in[:, :])
                W1T_sb = sp.tile([128, 64], f32)
                nc.sync.dma_start(out=W1T_sb[:], in_=W1T_in[:, :])
                b1_sb = sp.tile([1, 64], f32)
                nc.sync.dma_start(out=b1_sb[:], in_=b1_in[:, :])
                W3T_sb = sp.tile([64, 1], f32)
                nc.sync.dma_start(out=W3T_sb[:], in_=W3T_in[:, :])
                b3_sb = sp.tile([1, 1], f32)
                nc.sync.dma_start(out=b3_sb[:], in_=b3_in[:, :])

                hs = sp.tile([128, PMAX * F], f32)     # [graph, slot, feat]
                for s0 in range(0, PMAX, 8):
                    nc.gpsimd.dma_gather(
                        hs[:].rearrange("p (s f) -> p s f", f=F)[:, s0:s0 + 8, :],
                        h_dram[:, :],
                        s2s_idx_sb[:, s0 * 8:(s0 + 8) * 8],
                        8 * 128, 8 * 128, F, single_packet=False)

                qs = sp.tile([128, 2 * F], f32)
                nc.vector.memset(qs[:], 0)
                hh = sp.tile([128, F], f32)
                nc.vector.memset(hh[:], 0)
                cc = sp.tile([128, F], f32)
                nc.vector.memset(cc[:], 0)
                SCH = 40

                def transpose_to(dst_sb, src_ap, width):
                    tp = ps2.tile([128, 128], f32, tag="tp2")
                    nc.tensor.transpose(out=tp[:width, :], in_=src_ap,
                                        identity=ident[:])
                    nc.vector.tensor_copy(out=dst_sb[:width, :], in_=tp[:width, :])

                for _step in range(STEPS):
                    qsT = sw.tile([128, 2 * 128], f32, tag="qsT")
                    transpose_to(qsT[:, 0:128], qs[:, 0:F], 128)
                    transpose_to(qsT[:, 128:256], qs[:, F:2 * F], 128)
                    hhT = sw.tile([128, 128], f32, tag="hhT")
                    transpose_to(hhT, hh[:], 128)
                    gates = ps2.tile([128, 4 * F], f32, tag="gates")
                    nc.tensor.matmul(out=gates[:], lhsT=qsT[:, 0:128],
                                     rhs=WihT_sb[:, :4 * F], start=True, stop=False)
                    nc.tensor.matmul(out=gates[:], lhsT=qsT[:, 128:256],
                                     rhs=WihT_sb[:, 4 * F:], start=False, stop=False)
                    nc.tensor.matmul(out=gates[:], lhsT=hhT[:],
                                     rhs=WhhT_sb[:], start=False, stop=False)
                    nc.tensor.matmul(out=gates[:], lhsT=ones1[:],
                                     rhs=bsum[:], start=False, stop=True)
                    ig = sw.tile([128, F], f32, tag="ig")
                    nc.scalar.activation(out=ig[:], in_=gates[:, 0:F], func=AF.Sigmoid)
                    fg = sw.tile([128, F], f32, tag="fg")
                    nc.scalar.activation(out=fg[:], in_=gates[:, F:2 * F], func=AF.Sigmoid)
                    gg = sw.tile([128, F], f32, tag="gg")
                    nc.scalar.activation(out=gg[:], in_=gates[:, 2 * F:3 * F], func=AF.Tanh)
                    og = sw.tile([128, F], f32, tag="og")
                    nc.scalar.activation(out=og[:], in_=gates[:, 3 * F:4 * F], func=AF.Sigmoid)
                    t1 = sw.tile([128, F], f32, tag="t1")
                    nc.vector.tensor_tensor(out=t1[:], in0=fg[:], in1=cc[:], op=ALU.mult)
                    t2 = sw.tile([128, F], f32, tag="t2")
                    nc.vector.tensor_tensor(out=t2[:], in0=ig[:], in1=gg[:], op=ALU.mult)
                    nc.vector.tensor_tensor(out=cc[:], in0=t1[:], in1=t2[:], op=ALU.add)
                    tnc = sw.tile([128, F], f32, tag="tnc")
                    nc.scalar.activation(out=tnc[:], in_=cc[:], func=AF.Tanh)
                    nc.vector.tensor_tensor(out=hh[:], in0=og[:], in1=tnc[:], op=ALU.mult)

                    # attention (slot-chunked to bound SBUF)
                    e = sw.tile([128, PMAX], f32, tag="e")
                    for c0 in range(0, PMAX, SCH):
                        cw = min(SCH, PMAX - c0)
                        prodc = sw.tile([128, SCH * F], f32, tag="prodc")
                        nc.vector.tensor_tensor(
                            out=prodc[:, :cw * F].rearrange("p (s f) -> p s f", f=F),
                            in0=hs[:].rearrange("p (s f) -> p s f", f=F)[:, c0:c0 + cw, :],
                            in1=hh[:].rearrange("p (o f) -> p o f", o=1)
                            .to_broadcast([128, cw, F]),
                            op=ALU.mult)
                        nc.vector.tensor_reduce(
                            out=e[:, c0:c0 + cw],
                            in_=prodc[:, :cw * F].rearrange("p (s f) -> p s f", f=F),
                            axis=AX.X, op=ALU.add)
                    nc.vector.tensor_tensor(out=e[:], in0=e[:], in1=mask_sb[:],
                                            op=ALU.add)
                    negm = sw.tile([128, 1], f32, tag="negm")
                    nc.vector.tensor_reduce(out=negm[:], in_=e[:], axis=AX.X,
                                            op=ALU.max, negate=True)
                    ex = sw.tile([128, PMAX], f32, tag="ex")
                    nc.scalar.activation(out=ex[:], in_=e[:], func=AF.Exp,
                                         bias=negm[:, :], scale=1.0)
                    ssum = sw.tile([128, 1], f32, tag="ssum")
                    nc.vector.tensor_reduce(out=ssum[:], in_=ex[:], axis=AX.X,
                                            op=ALU.add)
                    rinv = sw.tile([128, 1], f32, tag="rinv")
                    nc.vector.reciprocal(out=rinv[:], in_=ssum[:])
                    a = sw.tile([128, PMAX], f32, tag="a")
                    nc.vector.tensor_tensor(out=a[:], in0=ex[:],
                                            in1=rinv[:].to_broadcast([128, PMAX]),
                                            op=ALU.mult)
                    r = sw.tile([128, F], f32, tag="r")
                    nc.vector.memset(r[:], 0)
                    for c0 in range(0, PMAX, SCH):
                        cw = min(SCH, PMAX - c0)
                        prodc = sw.tile([128, SCH * F], f32, tag="prodc")
                        nc.vector.tensor_tensor(
                            out=prodc[:, :cw * F].rearrange("p (s f) -> p s f", f=F),
                            in0=hs[:].rearrange("p (s f) -> p s f", f=F)[:, c0:c0 + cw, :],
                            in1=a[:, c0:c0 + cw].rearrange("p (s o) -> p s o", o=1)
                            .to_broadcast([128, cw, F]),
                            op=ALU.mult)
                        rq = sw.tile([128, F], f32, tag="rq")
                        nc.vector.tensor_reduce(
                            out=rq[:],
                            in_=prodc[:, :cw * F].rearrange("p (s f) -> p f s", f=F),
                            axis=AX.X, op=ALU.add)
                        nc.vector.tensor_tensor(out=r[:], in0=r[:], in1=rq[:],
                                                op=ALU.add)
                    nc.vector.tensor_copy(out=qs[:, 0:F], in_=hh[:])
                    nc.vector.tensor_copy(out=qs[:, F:2 * F], in_=r[:])

                # MLP head
                qsT = sw.tile([128, 2 * 128], f32, tag="qsT")
                transpose_to(qsT[:, 0:128], qs[:, 0:F], 128)
                transpose_to(qsT[:, 128:256], qs[:, F:2 * F], 128)
                z1p = ps2.tile([128, F], f32, tag="z1p")
                nc.tensor.matmul(out=z1p[:], lhsT=qsT[:, 0:128],
                                 rhs=W0T_sb[:, :F], start=True, stop=False)
                nc.tensor.matmul(out=z1p[:], lhsT=qsT[:, 128:256],
                                 rhs=W0T_sb[:, F:], start=False, stop=False)
                nc.tensor.matmul(out=z1p[:], lhsT=ones1[:], rhs=b0_sb[:],
                                 start=False, stop=True)
                z1 = sw.tile([128, F], f32, tag="z1")
                nc.scalar.activation(out=z1[:], in_=z1p[:], func=AF.Relu)
                z1T = sw.tile([128, 128], f32, tag="z1T")
                transpose_to(z1T, z1[:], 128)
                z2p = ps2.tile([128, 64], f32, tag="z2p")
                nc.tensor.matmul(out=z2p[:], lhsT=z1T[:], rhs=W1T_sb[:],
                                 start=True, stop=False)
                nc.tensor.matmul(out=z2p[:], lhsT=ones1[:], rhs=b1_sb[:],
                                 start=False, stop=True)
                z2 = sw.tile([128, 64], f32, tag="z2")
                nc.scalar.activation(out=z2[:], in_=z2p[:], func=AF.Relu)
                z2T = sw.tile([64, 128], f32, tag="z2T")
                tp = ps2.tile([128, 128], f32, tag="tp3")
                nc.tensor.transpose(out=tp[:64, :], in_=z2[:], identity=ident[:])
                nc.vector.tensor_copy(out=z2T[:, :], in_=tp[:64, :])
                z3p = ps2.tile([128, 1], f32, tag="z3p")
                nc.tensor.matmul(out=z3p[:], lhsT=z2T[:, :], rhs=W3T_sb[:],
                                 start=True, stop=False)
                nc.tensor.matmul(out=z3p[:], lhsT=ones1[:], rhs=b3_sb[:],
                                 start=False, stop=True)
                z3 = sw.tile([128, 1], f32, tag="z3")
                nc.vector.tensor_copy(out=z3[:], in_=z3p[:])
                nc.sync.dma_start(out=out[:, :], in_=z3[:GPC, :])

    nc.compile()
    return nc


def meta_cols(cfg, sched, c):
    tot = sum(kk * S for (_, kk, S) in sched[c]) * 128
    return tot // 16


# ---------------- runner ----------------

def _run(cfg, inputs, use_sim=False, trace=False):
    global LAST_EXEC_NS
    per_core, shared, meta = _prep(cfg, **inputs)
    nc = _build(cfg, meta, trunc=os.environ.get("GCN_TRUNC", "full"))
    in_maps = []
    for k in range(NCORES):
        m = dict(shared)
        m.update(per_core[k])
        m = {name: np.ascontiguousarray(v) for name, v in m.items()}
        in_maps.append(m)
    if use_sim:
        from concourse import bass_interp
        sim = bass_interp.MultiCoreSim(nc, NCORES)
        for k in range(NCORES):
            for name, v in in_maps[k].items():
                sim.cores[k].tensor(name)[:] = v
        sim.simulate(check_with_hw=False)
        outs = [np.array(sim.cores[k].mem_tensor("out")) for k in range(NCORES)]
    else:
        from concourse.bass_utils import run_bass_kernel_spmd
        if trace:
            _install_ntff_hook()
        res = run_bass_kernel_spmd(nc, in_maps, core_ids=list(range(NCORES)),
                                   trace=trace)
        LAST_EXEC_NS = res.exec_time_ns
        outs = [res.results[k]["out"] for k in range(NCORES)]
    return np.concatenate(outs, axis=0).astype(np.float32)


def kernel(**inputs) -> np.ndarray:
    trace = bool(os.environ.get("GCN_TRACE"))
    return _run(FULL, inputs, use_sim=False, trace=trace)
